# revision 45
# baseline (speedup 1.0000x reference)
"""NeRF-small MLP Bass kernel for Trainium2, 8-core data parallel.

Layout strategy: hidden-on-partitions, points-on-free-dim. Input x[N,6] is
loaded point-major (one fused DMA per 8192-pt super-tile), repacked 6->8
slots, and xbar-transposed to T4[128,512] bf16 where row 8q+c = channel c of
point-slab q. All 7 MLP layers run as K<=128, M<=128, N=512 matmuls with
block-diagonal "big" weight matrices processing two 512-point slabs (one
pair) per pass. Output staged as O[64,512] (row 4q+c' = out-channel c' of
slab q), xbar-transposed back to point-major and stored with
256B-contiguous-per-partition DMA.

Point permutation within a 2048-pt chunk: pt(p,q) = base + 16*p + q with
partition p, slot q - identical on input and output paths, so the final
output ordering matches the reference exactly.

Engine balance (per pair of slabs): PE runs 8 matmuls (the geo path
ws2[:,1:16] @ wc0[3:18] is composed offline into one h1-side matrix, so
there is no S2 pass and no g2 evacuation); PSUM evacuation is split
ACT (h0, c0, C3->O) / DVE (h1, c1h, c2h); SBUF-side work (input repack,
softplus polynomial) runs on the Pool engine, which cannot touch PSUM. Softplus avoids the Ln activation entirely
(sigma = relu(x) + t*R(t), t = exp(-|x|), R deg-3 poly) so every ACT
function used lives in one activation table and no ACT_TABLE_LOADs are
emitted. DMA: input loads + all XBAR transposes on SP (HWDGE), output
stores on Pool (SWDGE), keeping any one queue well under the compute time.
"""

import numpy as np
import ml_dtypes

N_TOTAL = 1048576
N_CORES = 8
NPC = N_TOTAL // N_CORES       # 131072 points per core
ST = 8192                      # points per super-tile (4 chunks of 2048)
N_ST = NPC // ST

# ln(1+t)/t ~= B0 + B1 t + B2 t^2 + B3 t^3 on [0,1] (Chebyshev fit,
# max abs err in t*R(t) is 5.1e-4 -- far inside the 2e-2 gate)
B0, B1, B2, B3 = 0.99930126, -0.48463524, 0.25187429, -0.0738988

_CACHE = {}


def _pack_weights(ws0, ws1, ws2, wc0, wc1, wc2, wc3):
    """Build block-diagonal 'big' stationary matrices (see module docstring)."""
    bf16 = ml_dtypes.bfloat16
    f32 = np.float32
    ws0, ws1, ws2, wc0, wc1, wc2, wc3 = [
        np.asarray(w, f32) for w in (ws0, ws1, ws2, wc0, wc1, wc2, wc3)
    ]
    w0big = np.zeros((128, 8 * 128), f32)
    c0vbig = np.zeros((128, 8 * 128), f32)
    for j in range(8):
        for q, off in ((2 * j, 0), (2 * j + 1, 64)):
            w0big[8 * q: 8 * q + 3, 128 * j + off: 128 * j + off + 64] = ws0
            c0vbig[8 * q + 3: 8 * q + 6, 128 * j + off: 128 * j + off + 64] = wc0[0:3]
    s1big = np.zeros((128, 128), f32)
    s1big[0:64, 0:64] = ws1
    s1big[64:128, 64:128] = ws1
    # geo path composed offline: geo @ wc0[3:18] = h1 @ (ws2[:,1:16] @ wc0[3:18])
    comp = ws2[:, 1:16] @ wc0[3:18]
    compbig = np.zeros((128, 128), f32)
    compbig[0:64, 0:64] = comp
    compbig[64:128, 64:128] = comp
    c1big = np.zeros((128, 128), f32)
    c1big[0:64, 0:64] = wc1
    c1big[64:128, 64:128] = wc1
    c2big = np.zeros((128, 128), f32)
    c2big[0:64, 0:64] = wc2
    c2big[64:128, 64:128] = wc2
    # c3big variant v (one per pair within a 4-pair group) scatters RGB into
    # rows 8v+{0..2}, 8v+{4..6} of a shared [32,512] accumulation bank; sigbig
    # passes sigma_raw = h1 @ ws2[:,0] through to rows 8v+3, 8v+7.
    c3big = np.zeros((128, 4 * 32), f32)
    sigbig = np.zeros((128, 4 * 32), f32)
    for v in range(4):
        base = 32 * v + 8 * v  # local col 8v inside variant v's [*,32] slice
        c3big[0:64, base: base + 3] = wc3[:, 0:3]
        c3big[64:128, base + 4: base + 7] = wc3[:, 0:3]
        sigbig[0:64, base + 3] = ws2[:, 0]
        sigbig[64:128, base + 7] = ws2[:, 0]
    return {
        "sigbig": sigbig.astype(bf16),
        "w0big": w0big.astype(bf16), "c0vbig": c0vbig.astype(bf16),
        "s1big": s1big.astype(bf16), "compbig": compbig.astype(bf16),
        "c1big": c1big.astype(bf16),
        "c2big": c2big.astype(bf16), "c3big": c3big.astype(bf16),
    }


def _build(npts):
    import concourse.mybir as mybir
    from concourse import bacc, tile

    dt = mybir.dt
    f32, bf16 = dt.float32, dt.bfloat16
    AF = mybir.ActivationFunctionType
    ALU = mybir.AluOpType

    nc = bacc.Bacc()
    x_in = nc.dram_tensor("x", [npts, 6], f32, kind="ExternalInput")
    out = nc.dram_tensor("out", [npts, 4], f32, kind="ExternalOutput")
    wshapes = {
        "w0big": [128, 8 * 128], "c0vbig": [128, 8 * 128],
        "s1big": [128, 128], "compbig": [128, 128],
        "c1big": [128, 128], "c2big": [128, 128], "c3big": [128, 128],
        "sigbig": [128, 128],
    }
    wdr = {n: nc.dram_tensor(n, s, bf16, kind="ExternalInput")
           for n, s in wshapes.items()}

    n_st = npts // ST
    # fused input view: one DMA per super-tile, [128, 4 chunks, 96 = 16q*6c]
    xvb = x_in.rearrange("(s k p q) c -> s p k (q c)", k=4, p=128, q=16)
    ov = out.rearrange("(u d k p q) c -> u k p d (q c)", d=2, k=4, p=128, q=16)

    with tile.TileContext(nc) as tc:
        with (
            tc.tile_pool(name="const", bufs=1) as constp,
            tc.tile_pool(name="xin", bufs=4) as xinp,
            tc.tile_pool(name="x8", bufs=1) as x8p,
            tc.tile_pool(name="t4", bufs=4) as t4p,
            tc.tile_pool(name="act", bufs=2) as actp,
            tc.tile_pool(name="ost", bufs=3) as ostp,
            tc.tile_pool(name="opm", bufs=2) as opmp,
            tc.tile_pool(name="psum", bufs=1, space="PSUM") as psump,
            tc.tile_pool(name="psh0", bufs=2, space="PSUM") as psh0p,
            tc.tile_pool(name="psh1", bufs=2, space="PSUM") as psh1p,
            tc.tile_pool(name="psc12", bufs=1, space="PSUM") as psc12p,
            tc.tile_pool(name="psc3", bufs=2, space="PSUM") as psc3p,
        ):
            W = {}
            for name, shp in wshapes.items():
                t = constp.tile(shp, bf16, tag=name)
                nc.gpsimd.dma_start(t[:], wdr[name][:])
                W[name] = t

            # four persistent repack buffers (rotate across super-tiles);
            # pad slots (q,6) (q,7) zeroed once and never rewritten, so the
            # zero weight rows always see finite data
            x8s = []
            for i in range(4):
                t = x8p.tile([128, 512], bf16, tag=f"x8_{i}")
                nc.gpsimd.memset(t[:], 0.0)
                x8s.append(t)

            def prefetch(s):
                # SWDGE casting load (HBM f32 -> SBUF bf16, contiguous), then
                # ONE 3D-AP repack spreads 6-channel groups into the 8-slot
                # transpose-source layout, then the XBAR builds t4 rows 8q+c.
                x6 = xinp.tile([128, 384], bf16, tag="x6")
                x8 = x8s[s % 4]
                t4 = t4p.tile([128, 512], bf16, tag="t4")
                with tc.high_priority():
                    nc.gpsimd.dma_start(x6.rearrange("p (k f) -> p k f", k=4), xvb[s])
                    nc.gpsimd.tensor_copy(
                        x8.rearrange("p (q c) -> p q c", c=8)[:, :, 0:6],
                        x6.rearrange("p (q c) -> p q c", c=6),
                    )
                    for k in range(4):
                        nc.sync.dma_start_transpose(
                            t4[:, 128 * k: 128 * (k + 1)],
                            x8[:, 128 * k: 128 * (k + 1)])
                return t4

            # Output stage of ST s runs during ST s+2 in three phases so no
            # engine FIFO head ever waits: the transposes' gate (O complete)
            # is 2 super-tiles old by emission; the softplus chain (sigma =
            # relu(x) + t*R(t), t = exp(-|x|), R deg-3 -- no Ln activation,
            # zero ACT table switches) follows the transposes by ~3 pairs,
            # and the final add + store 3 pairs later still.
            def out_phase0(u, O):
                opm = opmp.tile([128, 512], bf16, tag="opm")
                with tc.high_priority():
                    for k in range(4):
                        nc.sync.dma_start_transpose(
                            opm[:, 128 * k: 128 * (k + 1)], O[:, 128 * k: 128 * (k + 1)])
                return opm

            def out_phase1(u, O, opm):
                # sigma slots of BOTH super-tiles in one strided [128,8,16]
                # view (free-dim strides are legal, unlike partition strides)
                opm3 = opm.rearrange("p (kd q c) -> p kd q c", kd=8, q=16, c=4)[:, :, :, 3]
                def v(t):
                    return t.rearrange("p (a b) -> p a b", a=8)
                spa = opmp.tile([128, 128], f32, tag="spa")
                nc.scalar.activation(v(spa), opm3, AF.Abs)
                spt = opmp.tile([128, 128], f32, tag="spt")
                nc.scalar.activation(spt[:], spa[:], AF.Exp, scale=-1.0)
                t2 = opmp.tile([128, 128], f32, tag="t2")
                nc.gpsimd.tensor_tensor(t2[:], spt[:], spt[:], op=ALU.mult)
                pA = opmp.tile([128, 128], f32, tag="pA")
                nc.gpsimd.tensor_scalar(pA[:], spt[:], B1, B0, op0=ALU.mult, op1=ALU.add)
                pB = opmp.tile([128, 128], f32, tag="pB")
                nc.gpsimd.tensor_scalar(pB[:], spt[:], B3, B2, op0=ALU.mult, op1=ALU.add)
                pC = opmp.tile([128, 128], f32, tag="pC")
                nc.gpsimd.tensor_tensor(pC[:], pB[:], t2[:], op=ALU.mult)
                pD = opmp.tile([128, 128], f32, tag="pD")
                nc.gpsimd.tensor_tensor(pD[:], pA[:], pC[:], op=ALU.add)
                pP = opmp.tile([128, 128], f32, tag="pP")
                nc.gpsimd.tensor_tensor(pP[:], pD[:], spt[:], op=ALU.mult)
                pr = opmp.tile([128, 128], f32, tag="pr")
                nc.gpsimd.tensor_scalar_max(v(pr), opm3, 0.0)
                return opm, pP, pr

            def out_phase2(u, opm, pP, pr):
                # sigma written back into the bf16 staging tile; the store is
                # a SWDGE casting DMA (bf16 -> f32), one per chunk covering
                # both super-tiles of the pair
                opm3 = opm.rearrange("p (kd q c) -> p kd q c", kd=8, q=16, c=4)[:, :, :, 3]
                def v(t):
                    return t.rearrange("p (a b) -> p a b", a=8)
                nc.gpsimd.tensor_tensor(opm3, v(pP), v(pr), op=ALU.add)
                for k in range(4):
                    nc.gpsimd.dma_start(
                        ov[u, k],
                        opm[:, 128 * k: 128 * (k + 1)].rearrange("p (d f) -> p d f", d=2))

            t4q = [prefetch(0), prefetch(1), prefetch(2)]
            pending_out = []
            for s in range(n_st):
                t4 = t4q.pop(0)
                po = None
                if len(pending_out) == 2:
                    po = pending_out.pop(0)
                    po_opm = out_phase0(*po)
                if s + 3 < n_st:
                    t4q.append(prefetch(s + 3))

                if s % 2 == 0:
                    O = ostp.tile([128, 512], bf16, tag="O")
                drow = 64 * (s % 2)
                C3 = None

                # C3/SIG accumulation for pair j is emitted during pair j+1,
                # so the PE never reaches a matmul whose operand (c2h) was
                # produced by the immediately preceding matmul's evacuation
                def emit_c3(j, c2h, h1):
                    nonlocal C3
                    v = j % 4
                    if v == 0:
                        C3 = psc3p.tile([32, 512], f32, tag="C3")
                    nc.tensor.matmul(C3[:], W["c3big"][:, 32 * v: 32 * (v + 1)],
                                     c2h[:], start=(v == 0), stop=False)
                    nc.tensor.matmul(C3[:], W["sigbig"][:, 32 * v: 32 * (v + 1)],
                                     h1[:], start=False, stop=(v == 3))
                    if v == 3:
                        nc.scalar.activation(
                            O[drow + 32 * (j // 4): drow + 32 * (j // 4) + 32, :],
                            C3[:], AF.Copy)

                deferred = None
                for j in range(8):
                    if j == 4 and po is not None:
                        po_mid = out_phase1(*po, po_opm)
                    if j == 7 and po is not None:
                        out_phase2(po[0], *po_mid)
                        po = None
                    v = j % 4
                    H0 = psh0p.tile([128, 512], f32, tag="H0")
                    nc.tensor.matmul(H0[:], W["w0big"][:, 128 * j: 128 * (j + 1)],
                                     t4[:], start=True, stop=True)
                    h0 = actp.tile([128, 512], bf16, tag="h0")
                    nc.scalar.activation(h0[:], H0[:], AF.Relu)

                    H1 = psh1p.tile([128, 512], f32, tag="H1")
                    nc.tensor.matmul(H1[:], W["s1big"][:], h0[:], start=True, stop=True)
                    h1 = actp.tile([128, 512], bf16, tag="h1")
                    nc.vector.tensor_scalar_max(h1[:], H1[:], 0.0)

                    CV = psump.tile([128, 512], f32, tag="CV")
                    nc.tensor.matmul(CV[:], W["c0vbig"][:, 128 * j: 128 * (j + 1)],
                                     t4[:], start=True, stop=False)
                    nc.tensor.matmul(CV[:], W["compbig"][:], h1[:], start=False, stop=True)
                    c0 = actp.tile([128, 512], bf16, tag="c0")
                    nc.scalar.activation(c0[:], CV[:], AF.Relu)

                    C1 = psc12p.tile([128, 512], f32, tag="C12")
                    nc.tensor.matmul(C1[:], W["c1big"][:], c0[:], start=True, stop=True)
                    c1h = actp.tile([128, 512], bf16, tag="c1h")
                    nc.vector.tensor_scalar_max(c1h[:], C1[:], 0.0)

                    C2 = psc12p.tile([128, 512], f32, tag="C12")
                    nc.tensor.matmul(C2[:], W["c2big"][:], c1h[:], start=True, stop=True)
                    c2h = actp.tile([128, 512], bf16, tag="c2h")
                    nc.vector.tensor_scalar_max(c2h[:], C2[:], 0.0)

                    # 4-pair shared accumulation bank: pair j writes RGB into
                    # rows 8v+{0-2,4-6} and sigma_raw into rows 8v+{3,7}; the
                    # other rows get +0 from this pair's zero weight columns.
                    if deferred is not None:
                        emit_c3(*deferred)
                    deferred = (j, c2h, h1)
                emit_c3(*deferred)

                if s % 2 == 1:
                    pending_out.append((s // 2, O))
            for po in pending_out:
                opm = out_phase0(*po)
                mid = out_phase1(*po, opm)
                out_phase2(po[0], *mid)
    nc.compile()
    return nc


def _run(inputs, npts=NPC, trace=False, cores=N_CORES):
    from concourse import bass_utils

    key = npts
    if key not in _CACHE:
        _CACHE[key] = _build(npts)
    nc = _CACHE[key]
    wm = _pack_weights(inputs["ws0"], inputs["ws1"], inputs["ws2"],
                       inputs["wc0"], inputs["wc1"], inputs["wc2"], inputs["wc3"])
    x = np.ascontiguousarray(np.asarray(inputs["x"], np.float32))
    xs = x.reshape(cores, npts, 6)
    in_maps = [dict(wm, x=np.ascontiguousarray(xs[c])) for c in range(cores)]
    res = bass_utils.run_bass_kernel_spmd(
        nc, in_maps, core_ids=list(range(cores)), trace=trace)
    outs = np.concatenate([r["out"] for r in res.results], axis=0)
    return outs, res


def kernel(**inputs):
    out, _ = _run(inputs)
    return out.astype(np.float32)


# revision 46
# speedup vs baseline: 1.0353x; 1.0353x over previous
"""NeRF-small MLP Bass kernel for Trainium2, 8-core data parallel.

Layout strategy: hidden-on-partitions, points-on-free-dim. Input x[N,6] is
loaded point-major (one fused DMA per 8192-pt super-tile), repacked 6->8
slots, and xbar-transposed to T4[128,512] bf16 where row 8q+c = channel c of
point-slab q. All 7 MLP layers run as K<=128, M<=128, N=512 matmuls with
block-diagonal "big" weight matrices processing two 512-point slabs (one
pair) per pass. Output staged as O[64,512] (row 4q+c' = out-channel c' of
slab q), xbar-transposed back to point-major and stored with
256B-contiguous-per-partition DMA.

Point permutation within a 2048-pt chunk: pt(p,q) = base + 16*p + q with
partition p, slot q - identical on input and output paths, so the final
output ordering matches the reference exactly.

Engine balance (per pair of slabs): PE runs 8 matmuls (the geo path
ws2[:,1:16] @ wc0[3:18] is composed offline into one h1-side matrix, so
there is no S2 pass and no g2 evacuation); PSUM evacuation is split
ACT (h0, c0, C3->O) / DVE (h1, c1h, c2h); SBUF-side work (input repack,
softplus polynomial) runs on the Pool engine, which cannot touch PSUM. Softplus avoids the Ln activation entirely
(sigma = relu(x) + t*R(t), t = exp(-|x|), R deg-3 poly) so every ACT
function used lives in one activation table and no ACT_TABLE_LOADs are
emitted. DMA: input loads + all XBAR transposes on SP (HWDGE), output
stores on Pool (SWDGE), keeping any one queue well under the compute time.
"""

import numpy as np
import ml_dtypes

N_TOTAL = 1048576
N_CORES = 8
NPC = N_TOTAL // N_CORES       # 131072 points per core
ST = 8192                      # points per super-tile (4 chunks of 2048)
N_ST = NPC // ST

# ln(1+t)/t ~= B0 + B1 t + B2 t^2 + B3 t^3 on [0,1] (Chebyshev fit,
# max abs err in t*R(t) is 5.1e-4 -- far inside the 2e-2 gate)
B0, B1, B2, B3 = 0.99930126, -0.48463524, 0.25187429, -0.0738988

_CACHE = {}


def _pack_weights(ws0, ws1, ws2, wc0, wc1, wc2, wc3):
    """Build block-diagonal 'big' stationary matrices (see module docstring)."""
    bf16 = ml_dtypes.bfloat16
    f32 = np.float32
    ws0, ws1, ws2, wc0, wc1, wc2, wc3 = [
        np.asarray(w, f32) for w in (ws0, ws1, ws2, wc0, wc1, wc2, wc3)
    ]
    w0big = np.zeros((128, 8 * 128), f32)
    c0vbig = np.zeros((128, 8 * 128), f32)
    for j in range(8):
        for q, off in ((2 * j, 0), (2 * j + 1, 64)):
            w0big[8 * q: 8 * q + 3, 128 * j + off: 128 * j + off + 64] = ws0
            c0vbig[8 * q + 3: 8 * q + 6, 128 * j + off: 128 * j + off + 64] = wc0[0:3]
    s1big = np.zeros((128, 128), f32)
    s1big[0:64, 0:64] = ws1
    s1big[64:128, 64:128] = ws1
    # geo path composed offline: geo @ wc0[3:18] = h1 @ (ws2[:,1:16] @ wc0[3:18])
    comp = ws2[:, 1:16] @ wc0[3:18]
    compbig = np.zeros((128, 128), f32)
    compbig[0:64, 0:64] = comp
    compbig[64:128, 64:128] = comp
    c1big = np.zeros((128, 128), f32)
    c1big[0:64, 0:64] = wc1
    c1big[64:128, 64:128] = wc1
    c2big = np.zeros((128, 128), f32)
    c2big[0:64, 0:64] = wc2
    c2big[64:128, 64:128] = wc2
    # c3big variant v (one per pair within a 4-pair group) scatters RGB into
    # rows 8v+{0..2}, 8v+{4..6} of a shared [32,512] accumulation bank; sigbig
    # passes sigma_raw = h1 @ ws2[:,0] through to rows 8v+3, 8v+7.
    c3big = np.zeros((128, 4 * 32), f32)
    sigbig = np.zeros((128, 4 * 32), f32)
    for v in range(4):
        base = 32 * v + 8 * v  # local col 8v inside variant v's [*,32] slice
        c3big[0:64, base: base + 3] = wc3[:, 0:3]
        c3big[64:128, base + 4: base + 7] = wc3[:, 0:3]
        sigbig[0:64, base + 3] = ws2[:, 0]
        sigbig[64:128, base + 7] = ws2[:, 0]
    return {
        "sigbig": sigbig.astype(bf16),
        "w0big": w0big.astype(bf16), "c0vbig": c0vbig.astype(bf16),
        "s1big": s1big.astype(bf16), "compbig": compbig.astype(bf16),
        "c1big": c1big.astype(bf16),
        "c2big": c2big.astype(bf16), "c3big": c3big.astype(bf16),
    }


def _build(npts):
    import concourse.mybir as mybir
    from concourse import bacc, tile

    dt = mybir.dt
    f32, bf16 = dt.float32, dt.bfloat16
    AF = mybir.ActivationFunctionType
    ALU = mybir.AluOpType

    nc = bacc.Bacc()
    x_in = nc.dram_tensor("x", [npts, 6], f32, kind="ExternalInput")
    out = nc.dram_tensor("out", [npts, 4], f32, kind="ExternalOutput")
    wshapes = {
        "w0big": [128, 8 * 128], "c0vbig": [128, 8 * 128],
        "s1big": [128, 128], "compbig": [128, 128],
        "c1big": [128, 128], "c2big": [128, 128], "c3big": [128, 128],
        "sigbig": [128, 128],
    }
    wdr = {n: nc.dram_tensor(n, s, bf16, kind="ExternalInput")
           for n, s in wshapes.items()}

    n_st = npts // ST
    # fused input view: one DMA per super-tile, [128, 4 chunks, 96 = 16q*6c]
    xvb = x_in.rearrange("(s k p q) c -> s p k (q c)", k=4, p=128, q=16)
    ov = out.rearrange("(u d k p q) c -> u k p d (q c)", d=2, k=4, p=128, q=16)

    with tile.TileContext(nc) as tc:
        with (
            tc.tile_pool(name="const", bufs=1) as constp,
            tc.tile_pool(name="xin", bufs=4) as xinp,
            tc.tile_pool(name="x8", bufs=1) as x8p,
            tc.tile_pool(name="t4", bufs=4) as t4p,
            tc.tile_pool(name="act", bufs=2) as actp,
            tc.tile_pool(name="ost", bufs=3) as ostp,
            tc.tile_pool(name="opm", bufs=2) as opmp,
            tc.tile_pool(name="psum", bufs=1, space="PSUM") as psump,
            tc.tile_pool(name="psh0", bufs=2, space="PSUM") as psh0p,
            tc.tile_pool(name="psc3", bufs=2, space="PSUM") as psc3p,
        ):
            W = {}
            for name, shp in wshapes.items():
                t = constp.tile(shp, bf16, tag=name)
                nc.gpsimd.dma_start(t[:], wdr[name][:])
                W[name] = t

            # four persistent repack buffers (rotate across super-tiles);
            # pad slots (q,6) (q,7) zeroed once and never rewritten, so the
            # zero weight rows always see finite data
            x8s = []
            for i in range(4):
                t = x8p.tile([128, 512], bf16, tag=f"x8_{i}")
                nc.gpsimd.memset(t[:], 0.0)
                x8s.append(t)

            def prefetch(s):
                # SWDGE casting load (HBM f32 -> SBUF bf16, contiguous), then
                # ONE 3D-AP repack spreads 6-channel groups into the 8-slot
                # transpose-source layout, then the XBAR builds t4 rows 8q+c.
                x6 = xinp.tile([128, 384], bf16, tag="x6")
                x8 = x8s[s % 4]
                t4 = t4p.tile([128, 512], bf16, tag="t4")
                with tc.high_priority():
                    nc.gpsimd.dma_start(x6.rearrange("p (k f) -> p k f", k=4), xvb[s])
                    nc.gpsimd.tensor_copy(
                        x8.rearrange("p (q c) -> p q c", c=8)[:, :, 0:6],
                        x6.rearrange("p (q c) -> p q c", c=6),
                    )
                    for k in range(4):
                        nc.sync.dma_start_transpose(
                            t4[:, 128 * k: 128 * (k + 1)],
                            x8[:, 128 * k: 128 * (k + 1)])
                return t4

            # Output stage of ST s runs during ST s+2 in three phases so no
            # engine FIFO head ever waits: the transposes' gate (O complete)
            # is 2 super-tiles old by emission; the softplus chain (sigma =
            # relu(x) + t*R(t), t = exp(-|x|), R deg-3 -- no Ln activation,
            # zero ACT table switches) follows the transposes by ~3 pairs,
            # and the final add + store 3 pairs later still.
            def out_phase0(u, O):
                opm = opmp.tile([128, 512], bf16, tag="opm")
                with tc.high_priority():
                    for k in range(4):
                        nc.sync.dma_start_transpose(
                            opm[:, 128 * k: 128 * (k + 1)], O[:, 128 * k: 128 * (k + 1)])
                return opm

            def out_phase1(u, O, opm):
                # sigma slots of BOTH super-tiles in one strided [128,8,16]
                # view (free-dim strides are legal, unlike partition strides)
                opm3 = opm.rearrange("p (kd q c) -> p kd q c", kd=8, q=16, c=4)[:, :, :, 3]
                def v(t):
                    return t.rearrange("p (a b) -> p a b", a=8)
                spa = opmp.tile([128, 128], f32, tag="spa")
                nc.scalar.activation(v(spa), opm3, AF.Abs)
                spt = opmp.tile([128, 128], f32, tag="spt")
                nc.scalar.activation(spt[:], spa[:], AF.Exp, scale=-1.0)
                t2 = opmp.tile([128, 128], f32, tag="t2")
                nc.gpsimd.tensor_tensor(t2[:], spt[:], spt[:], op=ALU.mult)
                pA = opmp.tile([128, 128], f32, tag="pA")
                nc.gpsimd.tensor_scalar(pA[:], spt[:], B1, B0, op0=ALU.mult, op1=ALU.add)
                pB = opmp.tile([128, 128], f32, tag="pB")
                nc.gpsimd.tensor_scalar(pB[:], spt[:], B3, B2, op0=ALU.mult, op1=ALU.add)
                pC = opmp.tile([128, 128], f32, tag="pC")
                nc.gpsimd.tensor_tensor(pC[:], pB[:], t2[:], op=ALU.mult)
                pD = opmp.tile([128, 128], f32, tag="pD")
                nc.gpsimd.tensor_tensor(pD[:], pA[:], pC[:], op=ALU.add)
                pP = opmp.tile([128, 128], f32, tag="pP")
                nc.gpsimd.tensor_tensor(pP[:], pD[:], spt[:], op=ALU.mult)
                pr = opmp.tile([128, 128], f32, tag="pr")
                nc.gpsimd.tensor_scalar_max(v(pr), opm3, 0.0)
                return opm, pP, pr

            def out_phase2(u, opm, pP, pr):
                # sigma written back into the bf16 staging tile; the store is
                # a SWDGE casting DMA (bf16 -> f32), one per chunk covering
                # both super-tiles of the pair
                opm3 = opm.rearrange("p (kd q c) -> p kd q c", kd=8, q=16, c=4)[:, :, :, 3]
                def v(t):
                    return t.rearrange("p (a b) -> p a b", a=8)
                nc.gpsimd.tensor_tensor(opm3, v(pP), v(pr), op=ALU.add)
                for k in range(4):
                    nc.gpsimd.dma_start(
                        ov[u, k],
                        opm[:, 128 * k: 128 * (k + 1)].rearrange("p (d f) -> p d f", d=2))

            t4q = [prefetch(0), prefetch(1), prefetch(2)]
            pending_out = []
            for s in range(n_st):
                t4 = t4q.pop(0)
                po = None
                if len(pending_out) == 2:
                    po = pending_out.pop(0)
                    po_opm = out_phase0(*po)
                if s + 3 < n_st:
                    t4q.append(prefetch(s + 3))

                if s % 2 == 0:
                    O = ostp.tile([128, 512], bf16, tag="O")
                drow = 64 * (s % 2)
                C3 = None

                # C3/SIG accumulation for pair j is emitted during pair j+1,
                # so the PE never reaches a matmul whose operand (c2h) was
                # produced by the immediately preceding matmul's evacuation
                def emit_c3(j, c2h, h1):
                    nonlocal C3
                    v = j % 4
                    if v == 0:
                        C3 = psc3p.tile([32, 512], f32, tag="C3")
                    nc.tensor.matmul(C3[:], W["c3big"][:, 32 * v: 32 * (v + 1)],
                                     c2h[:], start=(v == 0), stop=False)
                    nc.tensor.matmul(C3[:], W["sigbig"][:, 32 * v: 32 * (v + 1)],
                                     h1[:], start=False, stop=(v == 3))
                    if v == 3:
                        nc.scalar.activation(
                            O[drow + 32 * (j // 4): drow + 32 * (j // 4) + 32, :],
                            C3[:], AF.Copy)

                deferred = None
                for j in range(8):
                    if j == 4 and po is not None:
                        po_mid = out_phase1(*po, po_opm)
                    if j == 7 and po is not None:
                        out_phase2(po[0], *po_mid)
                        po = None
                    v = j % 4
                    H0 = psh0p.tile([128, 512], f32, tag="H0")
                    nc.tensor.matmul(H0[:], W["w0big"][:, 128 * j: 128 * (j + 1)],
                                     t4[:], start=True, stop=True)
                    h0 = actp.tile([128, 512], bf16, tag="h0")
                    nc.scalar.activation(h0[:], H0[:], AF.Relu)

                    H1 = psump.tile([128, 512], f32, tag="H1")
                    nc.tensor.matmul(H1[:], W["s1big"][:], h0[:], start=True, stop=True)
                    h1 = actp.tile([128, 512], bf16, tag="h1")
                    nc.vector.tensor_scalar_max(h1[:], H1[:], 0.0)

                    CV = psump.tile([128, 512], f32, tag="CV")
                    nc.tensor.matmul(CV[:], W["c0vbig"][:, 128 * j: 128 * (j + 1)],
                                     t4[:], start=True, stop=False)
                    nc.tensor.matmul(CV[:], W["compbig"][:], h1[:], start=False, stop=True)
                    c0 = actp.tile([128, 512], bf16, tag="c0")
                    nc.scalar.activation(c0[:], CV[:], AF.Relu)

                    C1 = psump.tile([128, 512], f32, tag="C1")
                    nc.tensor.matmul(C1[:], W["c1big"][:], c0[:], start=True, stop=True)
                    c1h = actp.tile([128, 512], bf16, tag="c1h")
                    nc.vector.tensor_scalar_max(c1h[:], C1[:], 0.0)

                    C2 = psump.tile([128, 512], f32, tag="C2")
                    nc.tensor.matmul(C2[:], W["c2big"][:], c1h[:], start=True, stop=True)
                    c2h = actp.tile([128, 512], bf16, tag="c2h")
                    nc.vector.tensor_scalar_max(c2h[:], C2[:], 0.0)

                    # 4-pair shared accumulation bank: pair j writes RGB into
                    # rows 8v+{0-2,4-6} and sigma_raw into rows 8v+{3,7}; the
                    # other rows get +0 from this pair's zero weight columns.
                    if deferred is not None:
                        emit_c3(*deferred)
                    deferred = (j, c2h, h1)
                emit_c3(*deferred)

                if s % 2 == 1:
                    pending_out.append((s // 2, O))
            for po in pending_out:
                opm = out_phase0(*po)
                mid = out_phase1(*po, opm)
                out_phase2(po[0], *mid)
    nc.compile()
    return nc


def _run(inputs, npts=NPC, trace=False, cores=N_CORES):
    from concourse import bass_utils

    key = npts
    if key not in _CACHE:
        _CACHE[key] = _build(npts)
    nc = _CACHE[key]
    wm = _pack_weights(inputs["ws0"], inputs["ws1"], inputs["ws2"],
                       inputs["wc0"], inputs["wc1"], inputs["wc2"], inputs["wc3"])
    x = np.ascontiguousarray(np.asarray(inputs["x"], np.float32))
    xs = x.reshape(cores, npts, 6)
    in_maps = [dict(wm, x=np.ascontiguousarray(xs[c])) for c in range(cores)]
    res = bass_utils.run_bass_kernel_spmd(
        nc, in_maps, core_ids=list(range(cores)), trace=trace)
    outs = np.concatenate([r["out"] for r in res.results], axis=0)
    return outs, res


def kernel(**inputs):
    out, _ = _run(inputs)
    return out.astype(np.float32)


# revision 47
# speedup vs baseline: 1.0752x; 1.0386x over previous
"""NeRF-small MLP Bass kernel for Trainium2, 8-core data parallel.

Layout strategy: hidden-on-partitions, points-on-free-dim. Input x[N,6] is
loaded point-major (one fused DMA per 8192-pt super-tile), repacked 6->8
slots, and xbar-transposed to T4[128,512] bf16 where row 8q+c = channel c of
point-slab q. All 7 MLP layers run as K<=128, M<=128, N=512 matmuls with
block-diagonal "big" weight matrices processing two 512-point slabs (one
pair) per pass. Output staged as O[64,512] (row 4q+c' = out-channel c' of
slab q), xbar-transposed back to point-major and stored with
256B-contiguous-per-partition DMA.

Point permutation within a 2048-pt chunk: pt(p,q) = base + 16*p + q with
partition p, slot q - identical on input and output paths, so the final
output ordering matches the reference exactly.

Engine balance (per pair of slabs): PE runs 8 matmuls (the geo path
ws2[:,1:16] @ wc0[3:18] is composed offline into one h1-side matrix, so
there is no S2 pass and no g2 evacuation); PSUM evacuation is split
ACT (h0, c0, C3->O) / DVE (h1, c1h, c2h); SBUF-side work (input repack,
softplus polynomial) runs on the Pool engine, which cannot touch PSUM. Softplus avoids the Ln activation entirely
(sigma = relu(x) + t*R(t), t = exp(-|x|), R deg-3 poly) so every ACT
function used lives in one activation table and no ACT_TABLE_LOADs are
emitted. DMA: input loads + all XBAR transposes on SP (HWDGE), output
stores on Pool (SWDGE), keeping any one queue well under the compute time.
"""

import numpy as np
import ml_dtypes

N_TOTAL = 1048576
N_CORES = 8
NPC = N_TOTAL // N_CORES       # 131072 points per core
ST = 8192                      # points per super-tile (4 chunks of 2048)
N_ST = NPC // ST

# ln(1+t)/t ~= B0 + B1 t + B2 t^2 + B3 t^3 on [0,1] (Chebyshev fit,
# max abs err in t*R(t) is 5.1e-4 -- far inside the 2e-2 gate)
B0, B1, B2, B3 = 0.99930126, -0.48463524, 0.25187429, -0.0738988

_CACHE = {}


def _pack_weights(ws0, ws1, ws2, wc0, wc1, wc2, wc3):
    """Build block-diagonal 'big' stationary matrices (see module docstring)."""
    bf16 = ml_dtypes.bfloat16
    f32 = np.float32
    ws0, ws1, ws2, wc0, wc1, wc2, wc3 = [
        np.asarray(w, f32) for w in (ws0, ws1, ws2, wc0, wc1, wc2, wc3)
    ]
    w0big = np.zeros((128, 8 * 128), f32)
    c0vbig = np.zeros((128, 8 * 128), f32)
    for j in range(8):
        for q, off in ((2 * j, 0), (2 * j + 1, 64)):
            w0big[8 * q: 8 * q + 3, 128 * j + off: 128 * j + off + 64] = ws0
            c0vbig[8 * q + 3: 8 * q + 6, 128 * j + off: 128 * j + off + 64] = wc0[0:3]
    s1big = np.zeros((128, 128), f32)
    s1big[0:64, 0:64] = ws1
    s1big[64:128, 64:128] = ws1
    # geo path composed offline: geo @ wc0[3:18] = h1 @ (ws2[:,1:16] @ wc0[3:18])
    comp = ws2[:, 1:16] @ wc0[3:18]
    compbig = np.zeros((128, 128), f32)
    compbig[0:64, 0:64] = comp
    compbig[64:128, 64:128] = comp
    c1big = np.zeros((128, 128), f32)
    c1big[0:64, 0:64] = wc1
    c1big[64:128, 64:128] = wc1
    c2big = np.zeros((128, 128), f32)
    c2big[0:64, 0:64] = wc2
    c2big[64:128, 64:128] = wc2
    # c3big variant v (one per pair within a 4-pair group) scatters RGB into
    # rows 8v+{0..2}, 8v+{4..6} of a shared [32,512] accumulation bank; sigbig
    # passes sigma_raw = h1 @ ws2[:,0] through to rows 8v+3, 8v+7.
    c3big = np.zeros((128, 4 * 32), f32)
    sigbig = np.zeros((128, 4 * 32), f32)
    for v in range(4):
        base = 32 * v + 8 * v  # local col 8v inside variant v's [*,32] slice
        c3big[0:64, base: base + 3] = wc3[:, 0:3]
        c3big[64:128, base + 4: base + 7] = wc3[:, 0:3]
        sigbig[0:64, base + 3] = ws2[:, 0]
        sigbig[64:128, base + 7] = ws2[:, 0]
    return {
        "sigbig": sigbig.astype(bf16),
        "w0big": w0big.astype(bf16), "c0vbig": c0vbig.astype(bf16),
        "s1big": s1big.astype(bf16), "compbig": compbig.astype(bf16),
        "c1big": c1big.astype(bf16),
        "c2big": c2big.astype(bf16), "c3big": c3big.astype(bf16),
    }


def _build(npts):
    import concourse.mybir as mybir
    from concourse import bacc, tile

    dt = mybir.dt
    f32, bf16 = dt.float32, dt.bfloat16
    AF = mybir.ActivationFunctionType
    ALU = mybir.AluOpType

    nc = bacc.Bacc()
    x_in = nc.dram_tensor("x", [npts, 6], f32, kind="ExternalInput")
    out = nc.dram_tensor("out", [npts, 4], f32, kind="ExternalOutput")
    wshapes = {
        "w0big": [128, 8 * 128], "c0vbig": [128, 8 * 128],
        "s1big": [128, 128], "compbig": [128, 128],
        "c1big": [128, 128], "c2big": [128, 128], "c3big": [128, 128],
        "sigbig": [128, 128],
    }
    wdr = {n: nc.dram_tensor(n, s, bf16, kind="ExternalInput")
           for n, s in wshapes.items()}

    n_st = npts // ST
    # fused input view: one DMA per super-tile, [128, 4 chunks, 96 = 16q*6c]
    xvb = x_in.rearrange("(s k p q) c -> s p k (q c)", k=4, p=128, q=16)
    ov = out.rearrange("(u d k p q) c -> u k p d (q c)", d=2, k=4, p=128, q=16)

    with tile.TileContext(nc) as tc:
        with (
            tc.tile_pool(name="const", bufs=1) as constp,
            tc.tile_pool(name="xin", bufs=4) as xinp,
            tc.tile_pool(name="x8", bufs=1) as x8p,
            tc.tile_pool(name="t4", bufs=4) as t4p,
            tc.tile_pool(name="act", bufs=2) as actp,
            tc.tile_pool(name="ost", bufs=3) as ostp,
            tc.tile_pool(name="opm", bufs=2) as opmp,
            tc.tile_pool(name="psum", bufs=1, space="PSUM") as psump,
            tc.tile_pool(name="psh0", bufs=2, space="PSUM") as psh0p,
            tc.tile_pool(name="psc3", bufs=2, space="PSUM") as psc3p,
        ):
            W = {}
            for name, shp in wshapes.items():
                t = constp.tile(shp, bf16, tag=name)
                nc.gpsimd.dma_start(t[:], wdr[name][:])
                W[name] = t

            # four persistent repack buffers (rotate across super-tiles);
            # pad slots (q,6) (q,7) zeroed once and never rewritten, so the
            # zero weight rows always see finite data
            x8s = []
            for i in range(4):
                t = x8p.tile([128, 512], bf16, tag=f"x8_{i}")
                nc.gpsimd.memset(t[:], 0.0)
                x8s.append(t)

            def prefetch(s):
                # SWDGE casting load (HBM f32 -> SBUF bf16, contiguous), then
                # ONE 3D-AP repack spreads 6-channel groups into the 8-slot
                # transpose-source layout, then the XBAR builds t4 rows 8q+c.
                x6 = xinp.tile([128, 384], bf16, tag="x6")
                x8 = x8s[s % 4]
                t4 = t4p.tile([128, 512], bf16, tag="t4")
                with tc.high_priority():
                    nc.gpsimd.dma_start(x6.rearrange("p (k f) -> p k f", k=4), xvb[s])
                    nc.gpsimd.tensor_copy(
                        x8.rearrange("p (q c) -> p q c", c=8)[:, :, 0:6],
                        x6.rearrange("p (q c) -> p q c", c=6),
                    )
                    for k in range(4):
                        nc.sync.dma_start_transpose(
                            t4[:, 128 * k: 128 * (k + 1)],
                            x8[:, 128 * k: 128 * (k + 1)])
                return t4

            # Output stage of ST s runs during ST s+2 in three phases so no
            # engine FIFO head ever waits: the transposes' gate (O complete)
            # is 2 super-tiles old by emission; the softplus chain (sigma =
            # relu(x) + t*R(t), t = exp(-|x|), R deg-3 -- no Ln activation,
            # zero ACT table switches) follows the transposes by ~3 pairs,
            # and the final add + store 3 pairs later still.
            def out_phase0(u, O):
                opm = opmp.tile([128, 512], bf16, tag="opm")
                with tc.high_priority():
                    for k in range(4):
                        nc.sync.dma_start_transpose(
                            opm[:, 128 * k: 128 * (k + 1)], O[:, 128 * k: 128 * (k + 1)])
                return opm

            def out_phase1(u, O, opm):
                # sigma slots of BOTH super-tiles in one strided [128,8,16]
                # view (free-dim strides are legal, unlike partition strides)
                opm3 = opm.rearrange("p (kd q c) -> p kd q c", kd=8, q=16, c=4)[:, :, :, 3]
                def v(t):
                    return t.rearrange("p (a b) -> p a b", a=8)
                spa = opmp.tile([128, 128], f32, tag="spa")
                nc.scalar.activation(v(spa), opm3, AF.Abs)
                spt = opmp.tile([128, 128], f32, tag="spt")
                nc.scalar.activation(spt[:], spa[:], AF.Exp, scale=-1.0)
                t2 = opmp.tile([128, 128], f32, tag="t2")
                nc.gpsimd.tensor_tensor(t2[:], spt[:], spt[:], op=ALU.mult)
                pA = opmp.tile([128, 128], f32, tag="pA")
                nc.gpsimd.tensor_scalar(pA[:], spt[:], B1, B0, op0=ALU.mult, op1=ALU.add)
                pB = opmp.tile([128, 128], f32, tag="pB")
                nc.gpsimd.tensor_scalar(pB[:], spt[:], B3, B2, op0=ALU.mult, op1=ALU.add)
                pC = opmp.tile([128, 128], f32, tag="pC")
                nc.gpsimd.tensor_tensor(pC[:], pB[:], t2[:], op=ALU.mult)
                pD = opmp.tile([128, 128], f32, tag="pD")
                nc.gpsimd.tensor_tensor(pD[:], pA[:], pC[:], op=ALU.add)
                pP = opmp.tile([128, 128], f32, tag="pP")
                nc.gpsimd.tensor_tensor(pP[:], pD[:], spt[:], op=ALU.mult)
                pr = opmp.tile([128, 128], f32, tag="pr")
                nc.gpsimd.tensor_scalar_max(v(pr), opm3, 0.0)
                return opm, pP, pr

            def out_phase2(u, opm, pP, pr):
                # sigma written back into the bf16 staging tile; the store is
                # a SWDGE casting DMA (bf16 -> f32), one per chunk covering
                # both super-tiles of the pair
                opm3 = opm.rearrange("p (kd q c) -> p kd q c", kd=8, q=16, c=4)[:, :, :, 3]
                def v(t):
                    return t.rearrange("p (a b) -> p a b", a=8)
                nc.gpsimd.tensor_tensor(opm3, v(pP), v(pr), op=ALU.add)
                for k in range(4):
                    nc.gpsimd.dma_start(
                        ov[u, k],
                        opm[:, 128 * k: 128 * (k + 1)].rearrange("p (d f) -> p d f", d=2))

            t4q = [prefetch(0), prefetch(1), prefetch(2)]
            pending_out = []
            for s in range(n_st):
                t4 = t4q.pop(0)
                po = None
                if len(pending_out) == 2:
                    po = pending_out.pop(0)
                    po_opm = out_phase0(*po)
                if s + 3 < n_st:
                    t4q.append(prefetch(s + 3))

                if s % 2 == 0:
                    O = ostp.tile([128, 512], bf16, tag="O")
                drow = 64 * (s % 2)
                C3 = None

                # C3/SIG accumulation for pair j is emitted during pair j+1,
                # so the PE never reaches a matmul whose operand (c2h) was
                # produced by the immediately preceding matmul's evacuation
                def emit_c3(j, c2h, h1):
                    nonlocal C3
                    v = j % 4
                    if v == 0:
                        C3 = psc3p.tile([32, 512], f32, tag="C3")
                    nc.tensor.matmul(C3[:], W["c3big"][:, 32 * v: 32 * (v + 1)],
                                     c2h[:], start=(v == 0), stop=False)
                    nc.tensor.matmul(C3[:], W["sigbig"][:, 32 * v: 32 * (v + 1)],
                                     h1[:], start=False, stop=(v == 3))
                    if v == 3:
                        nc.scalar.activation(
                            O[drow + 32 * (j // 4): drow + 32 * (j // 4) + 32, :],
                            C3[:], AF.Copy)

                deferred = None
                for j in range(8):
                    if j == 4 and po is not None:
                        po_mid = out_phase1(*po, po_opm)
                    if j == 7 and po is not None:
                        out_phase2(po[0], *po_mid)
                        po = None
                    v = j % 4
                    H0 = psh0p.tile([128, 512], f32, tag="H0")
                    nc.tensor.matmul(H0[:], W["w0big"][:, 128 * j: 128 * (j + 1)],
                                     t4[:], start=True, stop=True)
                    h0 = actp.tile([128, 512], bf16, tag="h0")
                    nc.scalar.activation(h0[:], H0[:], AF.Relu)

                    H1 = psump.tile([128, 512], f32, tag="H1")
                    nc.tensor.matmul(H1[:], W["s1big"][:], h0[:], start=True, stop=True)
                    h1 = actp.tile([128, 512], bf16, tag="h1")
                    nc.vector.tensor_scalar_max(h1[:], H1[:], 0.0)

                    CV = psump.tile([128, 512], f32, tag="CV")
                    nc.tensor.matmul(CV[:], W["c0vbig"][:, 128 * j: 128 * (j + 1)],
                                     t4[:], start=True, stop=False)
                    nc.tensor.matmul(CV[:], W["compbig"][:], h1[:], start=False, stop=True)
                    c0 = actp.tile([128, 512], bf16, tag="c0")
                    nc.scalar.activation(c0[:], CV[:], AF.Relu)

                    C1 = psump.tile([128, 512], f32, tag="C1")
                    nc.tensor.matmul(C1[:], W["c1big"][:], c0[:], start=True, stop=True)
                    c1h = actp.tile([128, 512], bf16, tag="c1h")
                    nc.vector.tensor_scalar_max(c1h[:], C1[:], 0.0)

                    C2 = psump.tile([128, 512], f32, tag="C2")
                    nc.tensor.matmul(C2[:], W["c2big"][:], c1h[:], start=True, stop=True)
                    c2h = actp.tile([128, 512], bf16, tag="c2h")
                    if j % 2:
                        nc.scalar.activation(c2h[:], C2[:], AF.Relu)
                    else:
                        nc.vector.tensor_scalar_max(c2h[:], C2[:], 0.0)

                    # 4-pair shared accumulation bank: pair j writes RGB into
                    # rows 8v+{0-2,4-6} and sigma_raw into rows 8v+{3,7}; the
                    # other rows get +0 from this pair's zero weight columns.
                    if deferred is not None:
                        emit_c3(*deferred)
                    deferred = (j, c2h, h1)
                emit_c3(*deferred)

                if s % 2 == 1:
                    pending_out.append((s // 2, O))
            for po in pending_out:
                opm = out_phase0(*po)
                mid = out_phase1(*po, opm)
                out_phase2(po[0], *mid)
    nc.compile()
    return nc


def _run(inputs, npts=NPC, trace=False, cores=N_CORES):
    from concourse import bass_utils

    key = npts
    if key not in _CACHE:
        _CACHE[key] = _build(npts)
    nc = _CACHE[key]
    wm = _pack_weights(inputs["ws0"], inputs["ws1"], inputs["ws2"],
                       inputs["wc0"], inputs["wc1"], inputs["wc2"], inputs["wc3"])
    x = np.ascontiguousarray(np.asarray(inputs["x"], np.float32))
    xs = x.reshape(cores, npts, 6)
    in_maps = [dict(wm, x=np.ascontiguousarray(xs[c])) for c in range(cores)]
    res = bass_utils.run_bass_kernel_spmd(
        nc, in_maps, core_ids=list(range(cores)), trace=trace)
    outs = np.concatenate([r["out"] for r in res.results], axis=0)
    return outs, res


def kernel(**inputs):
    out, _ = _run(inputs)
    return out.astype(np.float32)
